# revision 14
# baseline (speedup 1.0000x reference)
"""CenterLoss kernel for Trainium2 (raw Bass/Bacc, no Tile), 8-core
data-parallel.

Key algebraic insight: the reference builds the full [B, C] squared-
distance matrix and masks it with one-hot(labels), so only
distmat[i, labels[i]] survives.  The loss is therefore

    loss = (1/B) * sum_i || x_i - centers[labels[i]] ||^2

which needs only a gather of each sample's center row (indirect DMA), not
the [4096, 10000] matmul.

Sharding: data-parallel over the batch.  Each of the 8 cores gets 512
samples (x shard + labels shard) and the full replicated centers table in
DRAM; it gathers its 512 center rows, computes per-partition partial sums
of ||x - c||^2 on device, and the host reduces the 8x[128,4] partials.

v5 vs v1 (21997 ns): two structural fixes driven by the DMA-queue records
in the profile.
(1) SDMA engines drain ALL pending HWDGE (Q1) work before touching the
SWDGE gather queue (Q0) — with x issued up front, the gathers' 1 MB sat
behind x's 1 MB and the first gather landed ~5.5us after its doorbell.
v5 issues x per chunk, each gated (via a GpSimd issue-marker semaphore)
behind the corresponding gather instruction's issue, so the two queues
interleave chunk-by-chunk: each gather lands ~1us after its doorbell and
chunk compute overlaps the remaining gather issue window (the Q7
descriptor-generation serialization, ~1.4us per gather, is the
critical path and is fully hidden except for the last chunk).
(2) The tail drops v1's PE partition-reduce + PSUM copy + Sync handoff:
Scalar's Square-activation accumulator columns go straight to DRAM
([128,4] f32) and the host does the final 512-element sum.

The gather structure itself stays 4 single-index-per-partition SWDGE
gathers: the HW ucode supports only one index per partition per
instruction (multi-column offset APs gather garbage), and the dma_gather
custom op costs an ~11us mlp library load per execution.

Per core (512 samples = 4 chunks x 128 partitions, interleaved layout:
chunk a holds samples {4p + a}, one per partition p):
  Sync   : labels DMA ([128,4] int32), x chunk 0, then x chunks 1-3 each
           after gather a's issue marker (2 KB contiguous strips)
  GpSimd : 4 indirect gathers (offset AP = labels column a), issue-marker
           sem_inc after each
  Vector : per-chunk subtract
  Scalar : per-chunk Square activation w/ accum -> partials[:,a]; final
           out DMA (HWDGE on ACT) of partials [128,4]
Host: sum(partials) / BATCH, summed over the 8 cores.

Manual semaphores; no Tile exit drain+butterfly+sem-clear (the bass entry
preamble clears sems, so re-execution stays safe).
"""

from contextlib import ExitStack

import numpy as np

import concourse.bacc as bacc
import concourse.bass as bass
from concourse import mybir
from concourse.bass_utils import run_bass_kernel_spmd

BATCH = 4096
NUM_CLASSES = 10000
FEAT_DIM = 512
N_CORES = 8
BPC = BATCH // N_CORES   # samples per core = 512
P = 128                  # SBUF partitions
CHUNKS = BPC // P        # 4 chunks of 128 samples per core

AF = mybir.AluOpType

_NC_CACHE = {}


def _build_bass():
    nc = bacc.Bacc(None, target_bir_lowering=False)

    x_in = nc.dram_tensor("x", [BPC, FEAT_DIM], mybir.dt.float32,
                          kind="ExternalInput")
    lab_in = nc.dram_tensor("labels", [BPC], mybir.dt.int32,
                            kind="ExternalInput")
    cen_in = nc.dram_tensor("centers", [NUM_CLASSES, FEAT_DIM],
                            mybir.dt.float32, kind="ExternalInput")
    out_t = nc.dram_tensor("out", [P, CHUNKS], mybir.dt.float32,
                           kind="ExternalOutput")

    with ExitStack() as ctx:
        ec = ctx.enter_context
        lab_sb = ec(nc.sbuf_tensor("lab_sb", [P, CHUNKS], mybir.dt.int32))
        xt = ec(nc.sbuf_tensor("xt", [P, CHUNKS * FEAT_DIM],
                               mybir.dt.float32))
        ct = ec(nc.sbuf_tensor("ct", [P, CHUNKS * FEAT_DIM],
                               mybir.dt.float32))
        dds = [ec(nc.sbuf_tensor(f"dd{a}", [P, FEAT_DIM], mybir.dt.float32))
               for a in range(CHUNKS)]
        sqs = [ec(nc.sbuf_tensor(f"sq{a}", [P, FEAT_DIM], mybir.dt.float32))
               for a in range(CHUNKS)]
        partials = ec(nc.sbuf_tensor("partials", [P, CHUNKS],
                                     mybir.dt.float32))
        s_lab = ec(nc.semaphore("s_lab"))
        s_xs = [ec(nc.semaphore(f"s_x{a}")) for a in range(CHUNKS)]
        s_cts = [ec(nc.semaphore(f"s_ct{a}")) for a in range(CHUNKS)]
        s_gi = ec(nc.semaphore("s_gi"))
        s_sub = ec(nc.semaphore("s_sub"))
        s_sq = ec(nc.semaphore("s_sq"))
        s_out = ec(nc.semaphore("s_out"))

        # x chunk a: rows {4p + a} -> xt column block a (2 KB strips)
        x_chunked = x_in[:].rearrange("(p a) f -> p a f", a=CHUNKS)

        # ---- Sync: labels first (gathers depend on them); x chunk 0 can go
        # immediately (it lands before gather 0's data); x chunks 1-3 wait
        # for gather a's ISSUE marker so their Q1 descriptors queue behind
        # gather a on the SDMA engines (engines drain one queue's pending
        # work before switching — unmarked x would starve all gathers). ----
        nc.sync.dma_start(
            out=lab_sb[:],
            in_=lab_in[:].rearrange("(p a) -> p a", a=CHUNKS),
        ).then_inc(s_lab, 16)
        for a in range(CHUNKS):
            if a > 0:
                nc.sync.wait_ge(s_gi, a)
            nc.sync.dma_start(
                out=xt[:, a * FEAT_DIM:(a + 1) * FEAT_DIM],
                in_=x_chunked[:, a, :],
            ).then_inc(s_xs[a], 16)

        # ---- GpSimd: 4 indirect gathers (SWDGE), one index per partition
        # per instruction (the only HW-supported indirect form), each
        # followed by an issue-marker increment for Sync ----
        nc.gpsimd.wait_ge(s_lab, 16)
        for a in range(CHUNKS):
            nc.gpsimd.indirect_dma_start(
                out=ct[:, a * FEAT_DIM:(a + 1) * FEAT_DIM],
                out_offset=None,
                in_=cen_in[:],
                in_offset=bass.IndirectOffsetOnAxis(
                    ap=lab_sb[:, a:a + 1], axis=0),
            ).then_inc(s_cts[a], 16)
            nc.gpsimd.sem_inc(s_gi, 1)

        # ---- Vector: per-chunk subtract ----
        for a in range(CHUNKS):
            sl = slice(a * FEAT_DIM, (a + 1) * FEAT_DIM)
            nc.vector.wait_ge(s_xs[a], 16)
            nc.vector.wait_ge(s_cts[a], 16)
            nc.vector.tensor_tensor(
                out=dds[a][:], in0=xt[:, sl], in1=ct[:, sl],
                op=AF.subtract).then_inc(s_sub, 1)

        # ---- Scalar: per-chunk square + free-dim accumulate (chunk lands
        # arrive ~1.4us apart, well above Scalar's ~1us ACTIVATE+READ pitch,
        # so Scalar never becomes the pacer) ----
        for a in range(CHUNKS):
            nc.scalar.wait_ge(s_sub, a + 1)
            nc.scalar.activation(
                out=sqs[a][:], in_=dds[a][:],
                func=mybir.ActivationFunctionType.Square,
                accum_out=partials[:, a:a + 1]).then_inc(s_sq, 1)

        # ---- Scalar: output DMA (HWDGE on ACT).  No completion wait: the
        # NRT exit barrier's Drain empties the HWDGE queue before execution
        # is reported complete. ----
        nc.scalar.wait_ge(s_sq, CHUNKS)
        nc.scalar.dma_start(out=out_t[:], in_=partials[:]).then_inc(s_out, 16)

    # Bacc defers register allocation + event-semaphore splitting to
    # compile(); the pjrt exec path serializes without calling it.
    nc.compile()
    return nc


def get_nc():
    if "nc" not in _NC_CACHE:
        _NC_CACHE["nc"] = _build_bass()
    return _NC_CACHE["nc"]


def kernel(x, labels, centers, _run_kwargs=None):
    x = np.ascontiguousarray(x, dtype=np.float32)
    labels = np.ascontiguousarray(labels).astype(np.int32)
    centers = np.ascontiguousarray(centers, dtype=np.float32)

    nc = get_nc()
    in_maps = [
        {
            "x": x[c * BPC:(c + 1) * BPC],
            "labels": labels[c * BPC:(c + 1) * BPC],
            "centers": centers,
        }
        for c in range(N_CORES)
    ]
    kwargs = _run_kwargs or {}
    out = run_bass_kernel_spmd(nc, in_maps, core_ids=list(range(N_CORES)),
                               **kwargs)
    # reduce the 8 per-core [128, 4] partial-sum tiles on the host
    total = np.float64(0.0)
    for r in out.results:
        total += np.asarray(r["out"], dtype=np.float64).sum()
    if kwargs:
        kernel.last_run = out
    return np.asarray(total / BATCH, dtype=np.float32)


# revision 15
# speedup vs baseline: 1.0476x; 1.0476x over previous
"""CenterLoss kernel for Trainium2 (raw Bass/Bacc, no Tile), 8-core
data-parallel.

Key algebraic insight: the reference builds the full [B, C] squared-
distance matrix and masks it with one-hot(labels), so only
distmat[i, labels[i]] survives.  The loss is therefore

    loss = (1/B) * sum_i || x_i - centers[labels[i]] ||^2

which needs only a gather of each sample's center row (indirect DMA), not
the [4096, 10000] matmul.

Sharding: data-parallel over the batch.  Each of the 8 cores gets 512
samples (x shard + labels shard) and the full replicated centers table in
DRAM; it gathers its 512 center rows, computes per-partition partial sums
of ||x - c||^2 on device, and the host reduces the 8x[128,4] partials.

v6 vs v1 (21997 ns): two fixes driven by the per-engine DMA records.
(1) The SDMA rings are DESCRIPTOR-overhead-bound here (~150-190 ns per
small descriptor), and each engine drains its pending HWDGE (x) work
before switching to the SWDGE gather queue.  v1 loaded x as two DMAs
with 4 KB per-partition strips (256 descriptors) which hogged the rings
until ~13.4us and pushed every gather land late.  v6 loads x as ONE DMA
with 8 KB contiguous per-partition strips (128 descriptors, the maximum
strip for this layout) so the rings are free for the gathers ~2.5us
earlier and each gather lands right behind its issue.
(2) The tail drops v1's PE partition-reduce + PSUM copy + Sync handoff:
Scalar's Square-activation accumulator columns go straight to DRAM
([128,4] f32) and the host does the final 512-element sum.

The gather structure stays 4 single-index-per-partition SWDGE gathers
(~1.4us apiece of Q7 descriptor generation, the critical path): the HW
ucode supports only one index per partition per instruction (multi-column
offset APs gather garbage), and the dma_gather custom op costs an ~11us
mlp library load per execution.

Per core (512 samples = 4 chunks x 128 partitions, interleaved layout:
chunk a holds samples {4p + a}, one per partition p):
  Sync   : labels DMA ([128,4] int32), then x as ONE DMA (partition p
           holds rows 4p..4p+3 = one 8 KB contiguous strip)
  GpSimd : 4 indirect gathers (offset AP = labels column a)
  Vector : per-chunk subtract
  Scalar : per-chunk Square activation w/ accum -> partials[:,a]; final
           out DMA (HWDGE on ACT) of partials [128,4]
Host: sum(partials) / BATCH, summed over the 8 cores.

Manual semaphores; no Tile exit drain+butterfly+sem-clear (the bass entry
preamble clears sems, so re-execution stays safe).
"""

from contextlib import ExitStack

import numpy as np

import concourse.bacc as bacc
import concourse.bass as bass
from concourse import mybir
from concourse.bass_utils import run_bass_kernel_spmd

BATCH = 4096
NUM_CLASSES = 10000
FEAT_DIM = 512
N_CORES = 8
BPC = BATCH // N_CORES   # samples per core = 512
P = 128                  # SBUF partitions
CHUNKS = BPC // P        # 4 chunks of 128 samples per core

AF = mybir.AluOpType

_NC_CACHE = {}


def _build_bass():
    nc = bacc.Bacc(None, target_bir_lowering=False)

    x_in = nc.dram_tensor("x", [BPC, FEAT_DIM], mybir.dt.float32,
                          kind="ExternalInput")
    lab_in = nc.dram_tensor("labels", [BPC], mybir.dt.int32,
                            kind="ExternalInput")
    cen_in = nc.dram_tensor("centers", [NUM_CLASSES, FEAT_DIM],
                            mybir.dt.float32, kind="ExternalInput")
    out_t = nc.dram_tensor("out", [P, CHUNKS], mybir.dt.float32,
                           kind="ExternalOutput")

    with ExitStack() as ctx:
        ec = ctx.enter_context
        lab_sb = ec(nc.sbuf_tensor("lab_sb", [P, CHUNKS], mybir.dt.int32))
        xt = ec(nc.sbuf_tensor("xt", [P, CHUNKS * FEAT_DIM],
                               mybir.dt.float32))
        ct = ec(nc.sbuf_tensor("ct", [P, CHUNKS * FEAT_DIM],
                               mybir.dt.float32))
        dds = [ec(nc.sbuf_tensor(f"dd{a}", [P, FEAT_DIM], mybir.dt.float32))
               for a in range(CHUNKS)]
        sqs = [ec(nc.sbuf_tensor(f"sq{a}", [P, FEAT_DIM], mybir.dt.float32))
               for a in range(CHUNKS)]
        partials = ec(nc.sbuf_tensor("partials", [P, CHUNKS],
                                     mybir.dt.float32))
        s_lab = ec(nc.semaphore("s_lab"))
        s_x = ec(nc.semaphore("s_x"))
        s_cts = [ec(nc.semaphore(f"s_ct{a}")) for a in range(CHUNKS)]
        s_sub = ec(nc.semaphore("s_sub"))
        s_sq = ec(nc.semaphore("s_sq"))
        s_out = ec(nc.semaphore("s_out"))

        # ---- Sync: labels first (gathers depend on them), then x as ONE
        # DMA — partition p holds rows 4p..4p+3 as a single 8 KB contiguous
        # strip (128 descriptors total; keeps the rings descriptor-light so
        # the gathers behind it drain early) ----
        nc.sync.dma_start(
            out=lab_sb[:],
            in_=lab_in[:].rearrange("(p a) -> p a", a=CHUNKS),
        ).then_inc(s_lab, 16)
        nc.sync.dma_start(
            out=xt[:],
            in_=x_in[:].rearrange("(p r) f -> p (r f)", r=CHUNKS),
        ).then_inc(s_x, 16)

        # ---- GpSimd: 4 indirect gathers (SWDGE), one index per partition
        # per instruction (the only HW-supported indirect form) ----
        nc.gpsimd.wait_ge(s_lab, 16)
        for a in range(CHUNKS):
            nc.gpsimd.indirect_dma_start(
                out=ct[:, a * FEAT_DIM:(a + 1) * FEAT_DIM],
                out_offset=None,
                in_=cen_in[:],
                in_offset=bass.IndirectOffsetOnAxis(
                    ap=lab_sb[:, a:a + 1], axis=0),
            ).then_inc(s_cts[a], 16)

        # ---- Vector: per-chunk subtract ----
        nc.vector.wait_ge(s_x, 16)
        for a in range(CHUNKS):
            sl = slice(a * FEAT_DIM, (a + 1) * FEAT_DIM)
            nc.vector.wait_ge(s_cts[a], 16)
            nc.vector.tensor_tensor(
                out=dds[a][:], in0=xt[:, sl], in1=ct[:, sl],
                op=AF.subtract).then_inc(s_sub, 1)

        # ---- Scalar: per-chunk square + free-dim accumulate (chunk lands
        # arrive >1us apart, above Scalar's ~1us ACTIVATE+READ pitch) ----
        for a in range(CHUNKS):
            nc.scalar.wait_ge(s_sub, a + 1)
            nc.scalar.activation(
                out=sqs[a][:], in_=dds[a][:],
                func=mybir.ActivationFunctionType.Square,
                accum_out=partials[:, a:a + 1]).then_inc(s_sq, 1)

        # ---- Scalar: output DMA (HWDGE on ACT).  No completion wait: the
        # NRT exit barrier's Drain empties the HWDGE queue before execution
        # is reported complete. ----
        nc.scalar.wait_ge(s_sq, CHUNKS)
        nc.scalar.dma_start(out=out_t[:], in_=partials[:]).then_inc(s_out, 16)

    # Bacc defers register allocation + event-semaphore splitting to
    # compile(); the pjrt exec path serializes without calling it.
    nc.compile()
    return nc


def get_nc():
    if "nc" not in _NC_CACHE:
        _NC_CACHE["nc"] = _build_bass()
    return _NC_CACHE["nc"]


def kernel(x, labels, centers, _run_kwargs=None):
    x = np.ascontiguousarray(x, dtype=np.float32)
    labels = np.ascontiguousarray(labels).astype(np.int32)
    centers = np.ascontiguousarray(centers, dtype=np.float32)

    nc = get_nc()
    in_maps = [
        {
            "x": x[c * BPC:(c + 1) * BPC],
            "labels": labels[c * BPC:(c + 1) * BPC],
            "centers": centers,
        }
        for c in range(N_CORES)
    ]
    kwargs = _run_kwargs or {}
    out = run_bass_kernel_spmd(nc, in_maps, core_ids=list(range(N_CORES)),
                               **kwargs)
    # reduce the 8 per-core [128, 4] partial-sum tiles on the host
    total = np.float64(0.0)
    for r in out.results:
        total += np.asarray(r["out"], dtype=np.float64).sum()
    if kwargs:
        kernel.last_run = out
    return np.asarray(total / BATCH, dtype=np.float32)


# revision 16
# speedup vs baseline: 1.0798x; 1.0308x over previous
"""CenterLoss kernel for Trainium2 (raw Bass/Bacc, no Tile), 8-core
data-parallel.

Key algebraic insight: the reference builds the full [B, C] squared-
distance matrix and masks it with one-hot(labels), so only
distmat[i, labels[i]] survives.  The loss is therefore

    loss = (1/B) * sum_i || x_i - centers[labels[i]] ||^2

which needs only a gather of each sample's center row (indirect DMA), not
the [4096, 10000] matmul.

Sharding: data-parallel over the batch.  Each of the 8 cores gets 512
samples (x shard + labels shard) and the full replicated centers table in
DRAM; it gathers its 512 center rows, computes per-partition partial sums
of ||x - c||^2 on device, and the host reduces the 8x[128,4] partials.

v6 vs v1 (21997 ns): two fixes driven by the per-engine DMA records.
(1) The SDMA rings are DESCRIPTOR-overhead-bound here (~150-190 ns per
small descriptor), and each engine drains its pending HWDGE (x) work
before switching to the SWDGE gather queue.  v1 loaded x as two DMAs
with 4 KB per-partition strips (256 descriptors) which hogged the rings
until ~13.4us and pushed every gather land late.  v6 loads x as ONE DMA
with 8 KB contiguous per-partition strips (128 descriptors, the maximum
strip for this layout) so the rings are free for the gathers ~2.5us
earlier and each gather lands right behind its issue.
(2) The tail drops v1's PE partition-reduce + PSUM copy + Sync handoff:
Scalar's Square-activation accumulator columns go straight to DRAM
([128,4] f32) and the host does the final 512-element sum.

The gather structure stays 4 single-index-per-partition SWDGE gathers
(~1.4us apiece of Q7 descriptor generation, the critical path): the HW
ucode supports only one index per partition per instruction (multi-column
offset APs gather garbage), and the dma_gather custom op costs an ~11us
mlp library load per execution.

Per core (512 samples = 4 chunks x 128 partitions, interleaved layout:
chunk a holds samples {4p + a}, one per partition p):
  Sync   : labels DMA ([128,4] int32), then x as ONE DMA (partition p
           holds rows 4p..4p+3 = one 8 KB contiguous strip)
  GpSimd : 4 indirect gathers (offset AP = labels column a)
  Vector : per-chunk subtract
  Scalar : per-chunk Square activation w/ accum -> partials[:,a]; final
           out DMA (HWDGE on ACT) of partials [128,4]
Host: sum(partials) / BATCH, summed over the 8 cores.

Manual semaphores; no Tile exit drain+butterfly+sem-clear (the bass entry
preamble clears sems, so re-execution stays safe).
"""

from contextlib import ExitStack

import numpy as np

import concourse.bacc as bacc
import concourse.bass as bass
from concourse import mybir
from concourse.bass_utils import run_bass_kernel_spmd

BATCH = 4096
NUM_CLASSES = 10000
FEAT_DIM = 512
N_CORES = 8
BPC = BATCH // N_CORES   # samples per core = 512
P = 128                  # SBUF partitions
CHUNKS = BPC // P        # 4 chunks of 128 samples per core

AF = mybir.AluOpType

_NC_CACHE = {}


def _build_bass():
    nc = bacc.Bacc(None, target_bir_lowering=False)

    x_in = nc.dram_tensor("x", [BPC, FEAT_DIM], mybir.dt.float32,
                          kind="ExternalInput")
    lab_in = nc.dram_tensor("labels", [BPC], mybir.dt.int32,
                            kind="ExternalInput")
    cen_in = nc.dram_tensor("centers", [NUM_CLASSES, FEAT_DIM],
                            mybir.dt.float32, kind="ExternalInput")
    out_t = nc.dram_tensor("out", [P, CHUNKS + 1], mybir.dt.float32,
                           kind="ExternalOutput")

    with ExitStack() as ctx:
        ec = ctx.enter_context
        lab_sb = ec(nc.sbuf_tensor("lab_sb", [P, CHUNKS], mybir.dt.int32))
        xt = ec(nc.sbuf_tensor("xt", [P, CHUNKS * FEAT_DIM],
                               mybir.dt.float32))
        ct = ec(nc.sbuf_tensor("ct", [P, CHUNKS * FEAT_DIM],
                               mybir.dt.float32))
        dds = [ec(nc.sbuf_tensor(f"dd{a}", [P, FEAT_DIM], mybir.dt.float32))
               for a in range(CHUNKS)]
        sqs = [ec(nc.sbuf_tensor(f"sq{a}", [P, FEAT_DIM], mybir.dt.float32))
               for a in range(CHUNKS)]
        partials = ec(nc.sbuf_tensor("partials", [P, CHUNKS + 1],
                                     mybir.dt.float32))
        warm = ec(nc.sbuf_tensor("warm", [1, 16], mybir.dt.float32))
        s_warm = ec(nc.semaphore("s_warm"))
        s_lab = ec(nc.semaphore("s_lab"))
        s_x = ec(nc.semaphore("s_x"))
        s_cts = [ec(nc.semaphore(f"s_ct{a}")) for a in range(CHUNKS)]
        s_sub = ec(nc.semaphore("s_sub"))
        s_v3 = ec(nc.semaphore("s_v3"))
        s_sq = ec(nc.semaphore("s_sq"))
        s_out = ec(nc.semaphore("s_out"))

        # ---- Sync: labels first (gathers depend on them), then x as ONE
        # DMA — partition p holds rows 4p..4p+3 as a single 8 KB contiguous
        # strip (128 descriptors total; keeps the rings descriptor-light so
        # the gathers behind it drain early) ----
        nc.sync.dma_start(
            out=lab_sb[:],
            in_=lab_in[:].rearrange("(p a) -> p a", a=CHUNKS),
        ).then_inc(s_lab, 16)
        nc.sync.dma_start(
            out=xt[:],
            in_=x_in[:].rearrange("(p r) f -> p (r f)", r=CHUNKS),
        ).then_inc(s_x, 16)

        # ---- GpSimd: warm the SWDGE path during the labels round trip
        # (the first SWDGE instruction otherwise pays ~150 ns extra), then
        # 4 indirect gathers, one index per partition per instruction (the
        # only HW-supported indirect form) ----
        nc.gpsimd.dma_start(out=warm[:], in_=cen_in[0:1, 0:16]).then_inc(
            s_warm, 16)
        nc.gpsimd.wait_ge(s_lab, 16)
        for a in range(CHUNKS):
            nc.gpsimd.indirect_dma_start(
                out=ct[:, a * FEAT_DIM:(a + 1) * FEAT_DIM],
                out_offset=None,
                in_=cen_in[:],
                in_offset=bass.IndirectOffsetOnAxis(
                    ap=lab_sb[:, a:a + 1], axis=0),
            ).then_inc(s_cts[a], 16)

        # ---- Vector: per-chunk subtract ----
        nc.vector.wait_ge(s_x, 16)
        for a in range(CHUNKS):
            sl = slice(a * FEAT_DIM, (a + 1) * FEAT_DIM)
            nc.vector.wait_ge(s_cts[a], 16)
            nc.vector.tensor_tensor(
                out=dds[a][:], in0=xt[:, sl], in1=ct[:, sl],
                op=AF.subtract).then_inc(s_sub, 1)

        # ---- Chunk 3 (last to land) is split to shorten the tail: Vector
        # squares+reduces its first half (tensor_tensor mult + tensor_reduce
        # into partials[:,4]) while Scalar squares the second half. ----
        a3 = CHUNKS - 1
        HB = FEAT_DIM // 2
        nc.vector.wait_ge(s_sub, CHUNKS)
        nc.vector.tensor_tensor(
            out=sqs[a3][:, 0:HB], in0=dds[a3][:, 0:HB], in1=dds[a3][:, 0:HB],
            op=AF.mult).then_inc(s_v3, 1)
        nc.vector.wait_ge(s_v3, 1)
        nc.vector.tensor_reduce(
            out=partials[:, CHUNKS:CHUNKS + 1], in_=sqs[a3][:, 0:HB],
            axis=mybir.AxisListType.X, op=AF.add).then_inc(s_v3, 1)

        # ---- Scalar: per-chunk square + free-dim accumulate (chunks 0-2
        # full, chunk 3 second half only) ----
        for a in range(CHUNKS):
            nc.scalar.wait_ge(s_sub, a + 1)
            if a < CHUNKS - 1:
                nc.scalar.activation(
                    out=sqs[a][:], in_=dds[a][:],
                    func=mybir.ActivationFunctionType.Square,
                    accum_out=partials[:, a:a + 1]).then_inc(s_sq, 1)
            else:
                nc.scalar.activation(
                    out=sqs[a][:, HB:], in_=dds[a][:, HB:],
                    func=mybir.ActivationFunctionType.Square,
                    accum_out=partials[:, a:a + 1]).then_inc(s_sq, 1)

        # ---- Scalar: output DMA (HWDGE on ACT).  No completion wait: the
        # NRT exit barrier's Drain empties the HWDGE queue before execution
        # is reported complete. ----
        nc.scalar.wait_ge(s_sq, CHUNKS)
        nc.scalar.wait_ge(s_v3, 2)
        nc.scalar.dma_start(out=out_t[:], in_=partials[:]).then_inc(s_out, 16)

    # Bacc defers register allocation + event-semaphore splitting to
    # compile(); the pjrt exec path serializes without calling it.
    nc.compile()
    return nc


def get_nc():
    if "nc" not in _NC_CACHE:
        _NC_CACHE["nc"] = _build_bass()
    return _NC_CACHE["nc"]


def kernel(x, labels, centers, _run_kwargs=None):
    x = np.ascontiguousarray(x, dtype=np.float32)
    labels = np.ascontiguousarray(labels).astype(np.int32)
    centers = np.ascontiguousarray(centers, dtype=np.float32)

    nc = get_nc()
    in_maps = [
        {
            "x": x[c * BPC:(c + 1) * BPC],
            "labels": labels[c * BPC:(c + 1) * BPC],
            "centers": centers,
        }
        for c in range(N_CORES)
    ]
    kwargs = _run_kwargs or {}
    out = run_bass_kernel_spmd(nc, in_maps, core_ids=list(range(N_CORES)),
                               **kwargs)
    # reduce the 8 per-core [128, 5] partial-sum tiles on the host
    total = np.float64(0.0)
    for r in out.results:
        total += np.asarray(r["out"], dtype=np.float64).sum()
    if kwargs:
        kernel.last_run = out
    return np.asarray(total / BATCH, dtype=np.float32)
